# revision 1
# baseline (speedup 1.0000x reference)
"""Bass/TRN2 kernel for nn_Classifier_3934190043587 (ragged two-level GRU classifier).

Strategy:
- Data parallel over events B=256 -> 32 events/core on 8 cores (jet dim stays
  with its event so the second GRU is core-local).
- Constituent GRU (J*B sequences, M=200 ragged steps, hidden 128):
  hidden-on-partition layout [128, 320]; per core the 320 sequences are sorted
  by length descending, so the active set at step t is a column prefix and a
  finished sequence's hidden column simply freezes (no gather needed).
  Per-rank lengths are padded to the max across cores (one shared SPMD
  program); padded steps carry a pad channel whose weight drives the
  update-gate preactivation to -30 => zc=sigmoid(-30)~0 => h frozen.
- Matmuls in float32r (TF32-ish, ~1e-4 rel err), everything else fp32.
- Gate math: PSUM accumulates x-side and h-side projections (+biases via a
  ones input channel), ACT does sigmoids/tanh, DVE does the 5 remaining
  elementwise ops per step.
- Transition con->jet: PE transposes h, multiplies by a 0/1 permutation
  matrix to regroup columns from length-sorted order to (j, event) order.
- Jet GRU (J=10 steps, hidden 32, batch 32/core) in the same style (full
  width, pad-frozen), then softmax([l0,l1]) = [sig(l0-l1), 1-sig(l0-l1)].
"""

import numpy as np

J, B, M = 10, 256, 200
DIM_JET, DIM_CON, EMB_DIM = 4, 3, 3
JET_OUT, CON_OUT, FIN_OUT = 64, 128, 32
NCORES = 8
EPB = B // NCORES          # events per core = 32
SEQ = J * EPB              # con sequences per core = 320
PADBIG = 50.0

last_results = None        # BassKernelResults of the most recent run (for test.py)
last_nc = None
last_in_maps = None


def _assign_events(L):
    """Greedy balance events over cores by total con work. L: [J, B] ints."""
    tot = (L + 1).sum(axis=0)                      # [B]
    order = np.argsort(-tot, kind="stable")
    loads = np.zeros(NCORES, dtype=np.int64)
    counts = np.zeros(NCORES, dtype=np.int64)
    events = [[] for _ in range(NCORES)]
    for b in order:
        c = min((c for c in range(NCORES) if counts[c] < EPB),
                key=lambda c: loads[c])
        events[c].append(int(b))
        loads[c] += tot[b]
        counts[c] += 1
    return [np.array(e, dtype=np.int64) for e in events]


def _prep(x_jet, x_con_kin, x_con_type, jet_mask, con_mask,
          W_jet, b_jet, emb, Wih_c, Whh_c, bih_c, bhh_c,
          Wih_f, Whh_f, bih_f, bhh_f, W_out, b_out):
    f32 = np.float32
    e = emb[x_con_type]                                   # [J,B,M,3]
    x6 = np.concatenate([x_con_kin, e], axis=-1).astype(f32)  # [J,B,M,6]
    L = con_mask.astype(np.int64)                         # [J,B]

    events = _assign_events(L)

    # per-core length-sorted con sequence order
    con_orders = []     # per core: array [SEQ, 2] of (j, b)
    Lsorted = np.zeros((NCORES, SEQ), dtype=np.int64)
    for c in range(NCORES):
        seqs = [(j, b) for b in events[c] for j in range(J)]
        Ls = np.array([L[j, b] for (j, b) in seqs])
        o = np.argsort(-Ls, kind="stable")
        con_orders.append(np.array(seqs, dtype=np.int64)[o])
        Lsorted[c] = Ls[o]

    # shared schedule: per rank the max length over cores
    Trank = 1 + Lsorted.max(axis=0)                       # [SEQ], non-increasing
    T = int(Trank[0])
    # fp32r matmuls require even moving sizes; round widths to multiples of 8
    # (over-width columns are pad-frozen so extra processing is a no-op)
    sched = np.array([min(SEQ, max(8, -8 * (-(int((Trank > t).sum())) // 8)))
                      for t in range(T)], dtype=np.int64)

    # per-core X tensor [T, 8, SEQ]: ch 0-5 data, 6 ones, 7 pad flag
    Xs = []
    for c in range(NCORES):
        co = con_orders[c]
        xs = x6[co[:, 0], co[:, 1]]                       # [SEQ, M, 6]
        X = np.zeros((T, 8, SEQ), dtype=f32)
        t_idx = np.arange(T)[:, None]                     # [T,1]
        Lc = Lsorted[c][None, :]                          # [1,SEQ]
        Tr = Trank[None, :]
        real = (t_idx <= Lc)                              # [T,SEQ]
        X[:, 0:6, :] = np.where(real[:, None, :],
                                xs.transpose(1, 2, 0)[:T], 0.0)
        X[:, 6, :] = 1.0
        X[:, 7, :] = (~real).astype(f32)
        Xs.append(X)

    # con weights, gates arranged [r | zc(negated) | n], biases on ones channel
    bias_c = (bih_c + bhh_c).astype(f32)                  # [384]
    wx = np.zeros((128, 512), dtype=f32)
    for m in range(4):
        r0 = 32 * m
        wx[r0:r0 + 6, 0:128] = Wih_c[:, 0:128]
        wx[r0 + 6, 0:128] = bias_c[0:128]
        wx[r0:r0 + 6, 128:256] = -Wih_c[:, 128:256]
        wx[r0 + 6, 128:256] = -bias_c[128:256]
        wx[r0 + 7, 128:256] = -PADBIG
        wx[r0:r0 + 6, 256:384] = Wih_c[:, 256:384]
        wx[r0 + 6, 256:384] = bih_c[256:384]
        wx[r0 + 6, 384:512] = bhh_c[256:384]
    whh = np.concatenate([Whh_c[:, 0:128], -Whh_c[:, 128:256],
                          Whh_c[:, 256:384]], axis=1).astype(f32)  # [128,384]

    # jet-side per-core tensors
    xjs, jpads, pmats = [], [], []
    for c in range(NCORES):
        xj = np.zeros((5, SEQ), dtype=f32)
        jp = np.zeros((2, SEQ), dtype=f32)
        P = np.zeros((SEQ, SEQ), dtype=f32)
        ev = events[c]
        for j in range(J):
            cols = slice(j * EPB, (j + 1) * EPB)
            xj[0:4, cols] = x_jet[j, ev].T
            xj[4, cols] = 1.0
            jp[0, cols] = 1.0
            jp[1, cols] = (j > jet_mask[ev]).astype(f32)
        # P[s, j*EPB+bb] = 1 iff con rank s is (j, ev[bb])
        pos = {}
        for bb, b in enumerate(ev):
            pos.update({(j, b): j * EPB + bb for j in range(J)})
        co = con_orders[c]
        for s in range(SEQ):
            P[s, pos[(int(co[s, 0]), int(co[s, 1]))]] = 1.0
        xjs.append(xj)
        jpads.append(jp)
        pmats.append(P)

    wjet = np.zeros((5, 64), dtype=f32)
    wjet[0:4] = W_jet
    wjet[4] = b_jet

    # jet GRU weights, gates [r | zc | n] each 32 wide
    def gates_f(Wrows):  # Wrows [K, 96] in torch order -> [r | -z | n]
        return np.concatenate([Wrows[:, 0:32], -Wrows[:, 32:64],
                               Wrows[:, 64:96]], axis=1).astype(f32)
    bias_f = (bih_f + bhh_f).astype(f32)
    wfhcp = gates_f(Wih_f[64:192])                        # [128, 96]
    wfhj = np.zeros((66, 96), dtype=f32)
    wfhj[0:64] = gates_f(Wih_f[0:64])
    wfhj[64, 0:32] = bias_f[0:32]
    wfhj[64, 32:64] = -bias_f[32:64]
    wfhj[64, 64:96] = bih_f[64:96]
    wfhj[65, 32:64] = -PADBIG
    whhf = np.zeros((33, 96), dtype=f32)
    whhf[0:32] = gates_f(Whh_f)
    whhf[32, 64:96] = bhh_f[64:96]
    whhfA = whhf[:, 0:64].copy()                          # [33, 64]
    whhfB = whhf[:, 64:96].copy()                         # [33, 32]

    wdiff = np.zeros((33, 1), dtype=f32)
    wdiff[0:32, 0] = W_out[:, 0] - W_out[:, 1]
    wdiff[32, 0] = b_out[0] - b_out[1]

    ident = np.eye(128, dtype=f32)

    shared = dict(wx=wx, whh=whh, wjet=wjet, wfhcp=wfhcp, wfhj=wfhj,
                  whhfA=whhfA, whhfB=whhfB, wdiff=wdiff, ident=ident)
    percore = [dict(xseq=np.ascontiguousarray(Xs[c]), xj=xjs[c],
                    jpad=jpads[c], pmat=pmats[c]) for c in range(NCORES)]
    return shared, percore, events, T, sched


def _build(T, sched):
    from contextlib import ExitStack
    from concourse import bass, bacc, tile, mybir

    f32 = mybir.dt.float32
    f32r = mybir.dt.float32r
    Act = mybir.ActivationFunctionType
    Alu = mybir.AluOpType

    nc = bacc.Bacc(None, target_bir_lowering=False, debug=False)

    d_xseq = nc.dram_tensor("xseq", [T, 8, SEQ], f32, kind="ExternalInput")
    d_wx = nc.dram_tensor("wx", [128, 512], f32, kind="ExternalInput")
    d_whh = nc.dram_tensor("whh", [128, 384], f32, kind="ExternalInput")
    d_xj = nc.dram_tensor("xj", [5, SEQ], f32, kind="ExternalInput")
    d_wjet = nc.dram_tensor("wjet", [5, 64], f32, kind="ExternalInput")
    d_jpad = nc.dram_tensor("jpad", [2, SEQ], f32, kind="ExternalInput")
    d_pmat = nc.dram_tensor("pmat", [SEQ, SEQ], f32, kind="ExternalInput")
    d_wfhcp = nc.dram_tensor("wfhcp", [128, 96], f32, kind="ExternalInput")
    d_wfhj = nc.dram_tensor("wfhj", [66, 96], f32, kind="ExternalInput")
    d_whhfA = nc.dram_tensor("whhfA", [33, 64], f32, kind="ExternalInput")
    d_whhfB = nc.dram_tensor("whhfB", [33, 32], f32, kind="ExternalInput")
    d_wdiff = nc.dram_tensor("wdiff", [33, 1], f32, kind="ExternalInput")
    d_ident = nc.dram_tensor("ident", [128, 128], f32, kind="ExternalInput")
    d_out0 = nc.dram_tensor("out0", [1, EPB], f32, kind="ExternalOutput")
    d_out1 = nc.dram_tensor("out1", [1, EPB], f32, kind="ExternalOutput")

    with tile.TileContext(nc) as tc, ExitStack() as top:
        const = top.enter_context(tc.tile_pool(name="const", bufs=1))
        state = top.enter_context(tc.tile_pool(name="state", bufs=1))

        wx = const.tile([128, 512], f32r)
        whh = const.tile([128, 384], f32r)
        wx_raw = const.tile([128, 512], f32)
        whh_raw = const.tile([128, 384], f32)
        nc.gpsimd.dma_start(wx_raw[:], d_wx[:])
        nc.gpsimd.dma_start(whh_raw[:], d_whh[:])
        nc.scalar.activation(wx[:], wx_raw[:], Act.Copy)
        nc.scalar.activation(whh[:], whh_raw[:], Act.Copy)

        h = state.tile([128, SEQ], f32r)
        h32 = h[:].bitcast(f32)
        zs = const.tile([128, SEQ], f32)
        nc.vector.memset(zs[:], 0.0)
        nc.scalar.activation(h[:], zs[:], Act.Copy)

        # ---- jet linear branch (independent of con GRU) ----
        hjaug = state.tile([66, SEQ], f32r)       # rows 0:64 elu, 64 ones, 65 pad
        xj = const.tile([5, SEQ], f32)
        wjet = const.tile([5, 64], f32)
        jraw = const.tile([2, SEQ], f32)
        nc.gpsimd.dma_start(xj[:], d_xj[:])
        nc.gpsimd.dma_start(wjet[:], d_wjet[:])
        nc.gpsimd.dma_start(jraw[:], d_jpad[:])
        nc.scalar.activation(hjaug[64:66, :], jraw[:], Act.Copy)
        with tc.tile_pool(name="pselu", bufs=1, space="PSUM") as pselu, \
             tc.tile_pool(name="elu", bufs=1) as elupool:
            jp = pselu.tile([64, SEQ], f32)
            nc.tensor.matmul(jp[:], wjet[:], xj[:], start=True, stop=True)
            t1 = elupool.tile([64, SEQ], f32)
            t2 = elupool.tile([64, SEQ], f32)
            t3 = elupool.tile([64, SEQ], f32)
            t4 = elupool.tile([64, SEQ], f32)
            nc.vector.tensor_scalar_min(t1[:], jp[:], 0.0)
            nc.scalar.activation(t2[:], t1[:], Act.Exp)
            nc.vector.tensor_scalar_add(t3[:], t2[:], -1.0)
            nc.scalar.activation(t4[:], jp[:], Act.Relu)
            nc.vector.tensor_add(hjaug[0:64, :], t3[:], t4[:])

        # ---- constituent GRU ----
        with tc.tile_pool(name="xin", bufs=3) as xin, \
             tc.tile_pool(name="gw", bufs=3) as gw, \
             tc.tile_pool(name="pscon", bufs=2, space="PSUM") as pscon:
            for c0 in range(0, T, 4):
                csteps = list(range(c0, min(c0 + 4, T)))
                xraw = xin.tile([128, SEQ], f32, tag="xr")
                for t in csteps:
                    m = t % 4
                    nc.sync.dma_start(xraw[32 * m:32 * m + 8, :], d_xseq[t])
                xt = xin.tile([128, SEQ], f32r, tag="x")
                nc.scalar.activation(xt[:], xraw[:], Act.Copy)
                for t in csteps:
                    m = t % 4
                    n = int(sched[t])
                    rz = pscon.tile([128, 1024], f32, tag="rz")
                    nb = pscon.tile([128, 1024], f32, tag="nb")
                    xs = xt[32 * m:32 * m + 8, 0:n]
                    hs = h[:, 0:n]
                    nc.tensor.matmul(rz[:, 0:n], wx[32 * m:32 * m + 8, 0:128], xs,
                                     start=True, stop=False,
                                     tile_position=(32 * m, 0))
                    nc.tensor.matmul(rz[:, 0:n], whh[:, 0:128], hs,
                                     start=False, stop=True)
                    nc.tensor.matmul(rz[:, 512:512 + n], wx[32 * m:32 * m + 8, 128:256],
                                     xs, start=True, stop=False,
                                     tile_position=(32 * m, 0))
                    nc.tensor.matmul(rz[:, 512:512 + n], whh[:, 128:256], hs,
                                     start=False, stop=True)
                    nc.tensor.matmul(nb[:, 0:n], wx[32 * m:32 * m + 8, 256:384], xs,
                                     start=True, stop=True,
                                     tile_position=(32 * m, 0))
                    nc.tensor.matmul(nb[:, 512:512 + n], whh[:, 256:384], hs,
                                     start=True, stop=False)
                    nc.tensor.matmul(nb[:, 512:512 + n], wx[32 * m:32 * m + 8, 384:512],
                                     xs, start=False, stop=True,
                                     tile_position=(32 * m, 0))

                    r = gw.tile([128, SEQ], f32, tag="r")
                    zc = gw.tile([128, SEQ], f32, tag="zc")
                    u = gw.tile([128, SEQ], f32, tag="u")
                    v = gw.tile([128, SEQ], f32, tag="v")
                    nn = gw.tile([128, SEQ], f32, tag="nn")
                    ee = gw.tile([128, SEQ], f32, tag="ee")
                    nc.scalar.activation(r[:, 0:n], rz[:, 0:n], Act.Sigmoid)
                    nc.scalar.activation(zc[:, 0:n], rz[:, 512:512 + n], Act.Sigmoid)
                    nc.vector.scalar_tensor_tensor(
                        u[:, 0:n], nb[:, 512:512 + n], 0.0, r[:, 0:n],
                        Alu.add, Alu.mult)
                    nc.vector.tensor_add(v[:, 0:n], u[:, 0:n], nb[:, 0:n])
                    nc.scalar.activation(nn[:, 0:n], v[:, 0:n], Act.Tanh)
                    hsl = h32[:, 0:n]
                    nc.vector.tensor_sub(v[:, 0:n], nn[:, 0:n], hsl)
                    nc.vector.tensor_mul(ee[:, 0:n], zc[:, 0:n], v[:, 0:n])
                    nc.vector.tensor_add(h[:, 0:n], hsl, ee[:, 0:n])

        # ---- transition: hcp[:, j*EPB+bb] = h_con of (j, ev[bb]) ----
        hcp = state.tile([128, SEQ], f32r)
        with tc.tile_pool(name="pstr", bufs=2, space="PSUM") as pstr, \
             tc.tile_pool(name="pshc", bufs=1, space="PSUM") as pshc, \
             tc.tile_pool(name="tr", bufs=1) as tr:
            ident = tr.tile([128, 128], f32)
            nc.gpsimd.dma_start(ident[:], d_ident[:])
            hcpp = pshc.tile([128, SEQ], f32)
            chunks = [(0, 128), (128, 128), (256, 64)]
            for k, (off, w) in enumerate(chunks):
                tp = pstr.tile([128, 128], f32, tag="tp")
                nc.tensor.transpose(tp[0:w, :], h32[:, off:off + w], ident[:])
                ht = tr.tile([128, 128], f32, tag=f"ht{k}")
                nc.vector.tensor_copy(ht[0:w, :], tp[0:w, :])
                pm = tr.tile([128, SEQ], f32, tag=f"pm{k}")
                nc.gpsimd.dma_start(pm[0:w, :], d_pmat[off:off + w, :])
                nc.tensor.matmul(hcpp[:], ht[0:w, :], pm[0:w, :],
                                 start=(k == 0), stop=(k == 2))
            nc.vector.tensor_copy(hcp[:], hcpp[:])

        # ---- jet GRU ----
        with tc.tile_pool(name="jw", bufs=1) as jw, \
             tc.tile_pool(name="psjet", bufs=2, space="PSUM") as psjet, \
             tc.tile_pool(name="jg", bufs=2) as jg:
            wfhcp = jw.tile([128, 96], f32r)
            wfhj = jw.tile([66, 96], f32r)
            whhfA = jw.tile([33, 64], f32r)
            whhfB = jw.tile([33, 32], f32r)
            wdiff = jw.tile([33, 1], f32r)
            for dst, dsrc in [(wfhcp, d_wfhcp), (wfhj, d_wfhj),
                              (whhfA, d_whhfA), (whhfB, d_whhfB),
                              (wdiff, d_wdiff)]:
                raw = jw.tile(list(dst.shape), f32, tag=f"raw_{dsrc.name}")
                nc.gpsimd.dma_start(raw[:], dsrc[:])
                nc.scalar.activation(dst[:], raw[:], Act.Copy)

            hf = jw.tile([33, EPB], f32r)
            hf32 = hf[:].bitcast(f32)
            zf = jw.tile([33, EPB], f32)
            nc.vector.memset(zf[0:32, :], 0.0)
            nc.vector.memset(zf[32:33, :], 1.0)
            nc.scalar.activation(hf[:], zf[:], Act.Copy)

            for j in range(J):
                cols = slice(j * EPB, (j + 1) * EPB)
                A = psjet.tile([32, 96], f32, tag="A")
                Bb = psjet.tile([32, 32], f32, tag="B")
                for g, (g0, g1) in enumerate([(0, 32), (32, 64), (64, 96)]):
                    nc.tensor.matmul(A[:, g0:g1], wfhcp[:, g0:g1], hcp[:, cols],
                                     start=(g == 0), stop=False)
                    nc.tensor.matmul(A[:, g0:g1], wfhj[:, g0:g1], hjaug[:, cols],
                                     start=False, stop=False)
                nc.tensor.matmul(A[:, 0:32], whhfA[:, 0:32], hf[:],
                                 start=False, stop=False)
                nc.tensor.matmul(A[:, 32:64], whhfA[:, 32:64], hf[:],
                                 start=False, stop=True)
                nc.tensor.matmul(Bb[:], whhfB[:], hf[:], start=True, stop=True)

                rj = jg.tile([32, 64], f32, tag="rj")
                uj = jg.tile([32, 32], f32, tag="uj")
                vj = jg.tile([32, 32], f32, tag="vj")
                nj = jg.tile([32, 32], f32, tag="nj")
                ej = jg.tile([32, 32], f32, tag="ej")
                nc.scalar.activation(rj[:], A[:, 0:64], Act.Sigmoid)
                nc.vector.scalar_tensor_tensor(uj[:], Bb[:], 0.0, rj[:, 0:32],
                                               Alu.add, Alu.mult)
                nc.vector.tensor_add(vj[:], uj[:], A[:, 64:96])
                nc.scalar.activation(nj[:], vj[:], Act.Tanh)
                hsl = hf32[0:32, :]
                nc.vector.tensor_sub(vj[:], nj[:], hsl)
                nc.vector.tensor_mul(ej[:], rj[:, 32:64], vj[:])
                nc.vector.tensor_add(hf[0:32, :], hsl, ej[:])

            C = psjet.tile([1, EPB], f32, tag="C")
            nc.tensor.matmul(C[:], wdiff[:], hf[:], start=True, stop=True)
            p0 = jg.tile([1, EPB], f32, tag="p0")
            p1 = jg.tile([1, EPB], f32, tag="p1")
            nc.scalar.activation(p0[:], C[:], Act.Sigmoid)
            nc.vector.tensor_scalar(p1[:], p0[:], -1.0, 1.0, Alu.mult, Alu.add)
            nc.sync.dma_start(d_out0[:], p0[:])
            nc.sync.dma_start(d_out1[:], p1[:])

    nc.compile()
    return nc


def kernel(x_jet, x_con_kin, x_con_type, jet_mask, con_mask,
           W_jet, b_jet, emb, Wih_c, Whh_c, bih_c, bhh_c,
           Wih_f, Whh_f, bih_f, bhh_f, W_out, b_out):
    global last_results, last_nc, last_in_maps
    from concourse.bass_utils import run_bass_kernel_spmd

    args = [np.asarray(a) for a in
            (x_jet, x_con_kin, x_con_type, jet_mask, con_mask, W_jet, b_jet,
             emb, Wih_c, Whh_c, bih_c, bhh_c, Wih_f, Whh_f, bih_f, bhh_f,
             W_out, b_out)]
    (x_jet, x_con_kin, x_con_type, jet_mask, con_mask, W_jet, b_jet, emb,
     Wih_c, Whh_c, bih_c, bhh_c, Wih_f, Whh_f, bih_f, bhh_f,
     W_out, b_out) = [a.astype(np.float32) if a.dtype.kind == "f" else a
                      for a in args]

    shared, percore, events, T, sched = _prep(
        x_jet, x_con_kin, x_con_type, jet_mask, con_mask, W_jet, b_jet, emb,
        Wih_c, Whh_c, bih_c, bhh_c, Wih_f, Whh_f, bih_f, bhh_f, W_out, b_out)

    nc = _build(T, sched)

    in_maps = [{**shared, **percore[c]} for c in range(NCORES)]
    last_nc, last_in_maps = nc, in_maps
    res = run_bass_kernel_spmd(nc, in_maps, core_ids=list(range(NCORES)))
    last_results = res

    probs = np.zeros((B, 2), dtype=np.float32)
    for c in range(NCORES):
        o0 = res.results[c]["out0"][0]
        o1 = res.results[c]["out1"][0]
        probs[events[c], 0] = o0
        probs[events[c], 1] = o1
    return probs



# revision 2
# speedup vs baseline: 1716.4278x; 1716.4278x over previous
"""Bass/TRN2 kernel v2 for nn_Classifier_3934190043587 (ragged two-level GRU).

Changes vs v1 (918ms wall / ~807us device):
- fp16 matmul inputs + gate tensors (PE 1 cyc/row at any width vs f32r's 4
  below 256; DVE 2x_1p mode on the fp16 elementwise ops; half the DMA bytes).
- k=2 interleaved sequence groups per core: two independent GRU chains
  (length-sorted round-robin split) pipeline across PE/ACT/DVE so the serial
  per-step dependency chain overlaps between groups.
- 6 matmuls/step/group: bhh_n folded into the n-gate via
  scalar_tensor_tensor((Whh_n h) + bhh_n) * r instead of a 7th matmul.
- X DMA in 128-partition chunks (4 steps x 32-row bands) instead of
  8-partition slices: ~10x fewer DMA-bottleneck ns.
- r/z sigmoid as ONE ACT instruction over a [2, w] access pattern.
"""

import numpy as np

J, B, M = 10, 256, 200
DIM_JET, DIM_CON, EMB_DIM = 4, 3, 3
JET_OUT, CON_OUT, FIN_OUT = 64, 128, 32
NCORES = 8
EPB = B // NCORES          # events per core = 32
SEQ = J * EPB              # con sequences per core = 320
NG = 2                     # interleaved groups
W = SEQ // NG              # sequences per group = 160
PADBIG = 50.0

last_results = None
last_nc = None
last_in_maps = None
last_T = None
last_sched = None


def _assign_events(L):
    """Greedy balance events over cores by total con work. L: [J, B] ints."""
    tot = (L + 1).sum(axis=0)                      # [B]
    order = np.argsort(-tot, kind="stable")
    loads = np.zeros(NCORES, dtype=np.int64)
    counts = np.zeros(NCORES, dtype=np.int64)
    events = [[] for _ in range(NCORES)]
    for b in order:
        c = min((c for c in range(NCORES) if counts[c] < EPB),
                key=lambda c: loads[c])
        events[c].append(int(b))
        loads[c] += tot[b]
        counts[c] += 1
    return [np.array(e, dtype=np.int64) for e in events]


def _prep(x_jet, x_con_kin, x_con_type, jet_mask, con_mask,
          W_jet, b_jet, emb, Wih_c, Whh_c, bih_c, bhh_c,
          Wih_f, Whh_f, bih_f, bhh_f, W_out, b_out):
    f16, f32 = np.float16, np.float32
    e = emb[x_con_type]                                   # [J,B,M,3]
    x6 = np.concatenate([x_con_kin, e], axis=-1).astype(f32)  # [J,B,M,6]
    L = con_mask.astype(np.int64)                         # [J,B]

    events = _assign_events(L)

    # per-core column order: groups A|B, each length-sorted desc (round-robin)
    con_orders = []     # per core: [SEQ, 2] of (j, b) in column order
    Lcols = np.zeros((NCORES, SEQ), dtype=np.int64)
    for c in range(NCORES):
        seqs = [(j, b) for b in events[c] for j in range(J)]
        Ls = np.array([L[j, b] for (j, b) in seqs])
        o = np.argsort(-Ls, kind="stable")
        oA, oB = o[0::2], o[1::2]
        oo = np.concatenate([oA, oB])
        con_orders.append(np.array(seqs, dtype=np.int64)[oo])
        Lcols[c] = Ls[oo]

    T = int(1 + Lcols.max())
    assert T <= M
    # per-group shared schedule: max active across cores, rounded to mult of 8
    sched = np.zeros((NG, T), dtype=np.int64)
    for g in range(NG):
        Lg = Lcols[:, g * W:(g + 1) * W]                  # [NCORES, W]
        for t in range(T):
            a = int((Lg >= t).sum(axis=1).max())
            sched[g, t] = min(W, max(8, 8 * ((a + 7) // 8)))

    # X tensor per core: [NCH, 128, SEQ] fp16, partition 32*(t%4)+ch
    NCH = (T + 3) // 4
    Xs = []
    for c in range(NCORES):
        co = con_orders[c]
        xs = x6[co[:, 0], co[:, 1]]                       # [SEQ, M, 6]
        X = np.zeros((NCH * 4, 8, SEQ), dtype=f32)
        t_idx = np.arange(T)[:, None]
        Lc = Lcols[c][None, :]
        real = (t_idx <= Lc)                              # [T,SEQ]
        X[:T, 0:6, :] = np.where(real[:, None, :],
                                 xs.transpose(1, 2, 0)[:T], 0.0)
        X[:T, 6, :] = 1.0
        X[:T, 7, :] = (~real).astype(f32)
        X = X.reshape(NCH, 4, 8, SEQ).reshape(NCH, 32, SEQ)
        # widen 32 -> 128 partitions: band m at rows 32m..32m+8
        Xw = np.zeros((NCH, 128, SEQ), dtype=f16)
        for m in range(4):
            Xw[:, 32 * m:32 * m + 8, :] = \
                X[:, 8 * m:8 * m + 8, :].astype(f16)
        Xs.append(Xw)

    # con weights fp16: gates [r | -z | n], biases via ones channel,
    # pad channel -PADBIG on -z; bhh_n applied later via stt scalar.
    bias_c = (bih_c + bhh_c).astype(f32)                  # [384]
    wx = np.zeros((128, 384), dtype=f32)
    for m in range(4):
        r0 = 32 * m
        wx[r0:r0 + 6, 0:128] = Wih_c[:, 0:128]
        wx[r0 + 6, 0:128] = bias_c[0:128]
        wx[r0:r0 + 6, 128:256] = -Wih_c[:, 128:256]
        wx[r0 + 6, 128:256] = -bias_c[128:256]
        wx[r0 + 7, 128:256] = -PADBIG
        wx[r0:r0 + 6, 256:384] = Wih_c[:, 256:384]
        wx[r0 + 6, 256:384] = bih_c[256:384]
    whh = np.concatenate([Whh_c[:, 0:128], -Whh_c[:, 128:256],
                          Whh_c[:, 256:384]], axis=1).astype(f32)  # [128,384]
    bhhn = bhh_c[256:384].astype(f32).reshape(128, 1)

    # jet-side per-core tensors
    xjs, jpads, pmats = [], [], []
    for c in range(NCORES):
        xj = np.zeros((5, SEQ), dtype=f32)
        jp = np.zeros((2, SEQ), dtype=f32)
        P = np.zeros((SEQ, SEQ), dtype=f32)
        ev = events[c]
        for j in range(J):
            cols = slice(j * EPB, (j + 1) * EPB)
            xj[0:4, cols] = x_jet[j, ev].T
            xj[4, cols] = 1.0
            jp[0, cols] = 1.0
            jp[1, cols] = (j > jet_mask[ev]).astype(f32)
        pos = {}
        for bb, b in enumerate(ev):
            pos.update({(j, b): j * EPB + bb for j in range(J)})
        co = con_orders[c]
        for s in range(SEQ):
            P[s, pos[(int(co[s, 0]), int(co[s, 1]))]] = 1.0
        xjs.append(xj.astype(f16))
        jpads.append(jp.astype(f16))
        pmats.append(P.astype(f16))

    wjet = np.zeros((5, 64), dtype=f32)
    wjet[0:4] = W_jet
    wjet[4] = b_jet

    # jet GRU weights, gates [r | -z | n] each 32 wide
    def gates_f(Wrows):
        return np.concatenate([Wrows[:, 0:32], -Wrows[:, 32:64],
                               Wrows[:, 64:96]], axis=1).astype(f32)
    bias_f = (bih_f + bhh_f).astype(f32)
    wfhcp = gates_f(Wih_f[64:192])                        # [128, 96]
    wfhj = np.zeros((66, 96), dtype=f32)
    wfhj[0:64] = gates_f(Wih_f[0:64])
    wfhj[64, 0:32] = bias_f[0:32]
    wfhj[64, 32:64] = -bias_f[32:64]
    wfhj[64, 64:96] = bih_f[64:96]
    wfhj[65, 32:64] = -PADBIG
    whhf = np.zeros((33, 96), dtype=f32)
    whhf[0:32] = gates_f(Whh_f)
    whhf[32, 64:96] = bhh_f[64:96]
    whhfA = whhf[:, 0:64].copy()                          # [33, 64]
    whhfB = whhf[:, 64:96].copy()                         # [33, 32]

    wdiff = np.zeros((33, 1), dtype=f32)
    wdiff[0:32, 0] = W_out[:, 0] - W_out[:, 1]
    wdiff[32, 0] = b_out[0] - b_out[1]

    ident = np.eye(128, dtype=f32)

    shared = dict(wx=wx.astype(f16), whh=whh.astype(f16), bhhn=bhhn,
                  wjet=wjet.astype(f16), wfhcp=wfhcp.astype(f16),
                  wfhj=wfhj.astype(f16), whhfA=whhfA.astype(f16),
                  whhfB=whhfB.astype(f16), wdiff=wdiff.astype(f16),
                  ident=ident)
    percore = [dict(xseq=np.ascontiguousarray(Xs[c]), xj=xjs[c],
                    jpad=jpads[c], pmat=pmats[c]) for c in range(NCORES)]
    return shared, percore, events, T, sched


def _build(T, sched, reps=1):
    from contextlib import ExitStack
    from concourse import bass, bacc, tile, mybir

    f16 = mybir.dt.float16
    f32 = mybir.dt.float32
    Act = mybir.ActivationFunctionType
    Alu = mybir.AluOpType
    NCH = (T + 3) // 4

    nc = bacc.Bacc(None, target_bir_lowering=False, debug=False)

    d_xseq = nc.dram_tensor("xseq", [NCH, 128, SEQ], f16, kind="ExternalInput")
    d_wx = nc.dram_tensor("wx", [128, 384], f16, kind="ExternalInput")
    d_whh = nc.dram_tensor("whh", [128, 384], f16, kind="ExternalInput")
    d_bhhn = nc.dram_tensor("bhhn", [128, 1], f32, kind="ExternalInput")
    d_xj = nc.dram_tensor("xj", [5, SEQ], f16, kind="ExternalInput")
    d_wjet = nc.dram_tensor("wjet", [5, 64], f16, kind="ExternalInput")
    d_jpad = nc.dram_tensor("jpad", [2, SEQ], f16, kind="ExternalInput")
    d_pmat = nc.dram_tensor("pmat", [SEQ, SEQ], f16, kind="ExternalInput")
    d_wfhcp = nc.dram_tensor("wfhcp", [128, 96], f16, kind="ExternalInput")
    d_wfhj = nc.dram_tensor("wfhj", [66, 96], f16, kind="ExternalInput")
    d_whhfA = nc.dram_tensor("whhfA", [33, 64], f16, kind="ExternalInput")
    d_whhfB = nc.dram_tensor("whhfB", [33, 32], f16, kind="ExternalInput")
    d_wdiff = nc.dram_tensor("wdiff", [33, 1], f16, kind="ExternalInput")
    d_ident = nc.dram_tensor("ident", [128, 128], f32, kind="ExternalInput")
    d_out0 = nc.dram_tensor("out0", [1, EPB], f32, kind="ExternalOutput")
    d_out1 = nc.dram_tensor("out1", [1, EPB], f32, kind="ExternalOutput")

    with tile.TileContext(nc) as tc, ExitStack() as top:
        const = top.enter_context(tc.tile_pool(name="const", bufs=1))
        state = top.enter_context(tc.tile_pool(name="state", bufs=1))

        wx = const.tile([128, 384], f16)
        whh = const.tile([128, 384], f16)
        bhhn = const.tile([128, 1], f32)
        nc.gpsimd.dma_start(wx[:], d_wx[:])
        nc.gpsimd.dma_start(whh[:], d_whh[:])
        nc.gpsimd.dma_start(bhhn[:], d_bhhn[:])

        xj = const.tile([5, SEQ], f16)
        wjet = const.tile([5, 64], f16)
        jraw = const.tile([2, SEQ], f16)
        nc.gpsimd.dma_start(xj[:], d_xj[:])
        nc.gpsimd.dma_start(wjet[:], d_wjet[:])
        nc.gpsimd.dma_start(jraw[:], d_jpad[:])

      # repeated body (reps>1 only for marginal HW timing in test.py)
      for rep in range(reps):
        h = [state.tile([128, W], f16, tag=f"h{g}", name=f"h{g}")
             for g in range(NG)]
        for g in range(NG):
            nc.vector.memset(h[g][:], 0.0)

        # ---- jet linear branch (independent of con GRU) ----
        hjaug = state.tile([66, SEQ], f16, tag="hjaug", name="hjaug")
        nc.scalar.activation(hjaug[64:66, :], jraw[:], Act.Copy)
        with tc.tile_pool(name=f"pselu{rep}", bufs=1, space="PSUM") as pselu, \
             tc.tile_pool(name=f"elu{rep}", bufs=1) as elupool:
            jp = pselu.tile([64, SEQ], f32)
            nc.tensor.matmul(jp[:], wjet[:], xj[:], start=True, stop=True)
            t1 = elupool.tile([64, SEQ], f32)
            t2 = elupool.tile([64, SEQ], f32)
            t3 = elupool.tile([64, SEQ], f32)
            t4 = elupool.tile([64, SEQ], f32)
            nc.vector.tensor_scalar_min(t1[:], jp[:], 0.0)
            nc.scalar.activation(t2[:], t1[:], Act.Exp)
            nc.vector.tensor_scalar_add(t3[:], t2[:], -1.0)
            nc.scalar.activation(t4[:], jp[:], Act.Relu)
            nc.vector.tensor_add(hjaug[0:64, :], t3[:], t4[:])

        # ---- constituent GRU ----
        with tc.tile_pool(name=f"xin{rep}", bufs=3) as xin, \
             tc.tile_pool(name=f"gw{rep}", bufs=2) as gw, \
             tc.tile_pool(name=f"pscon{rep}", bufs=2, space="PSUM") as pscon:
            for c in range(NCH):
                xt = xin.tile([128, SEQ], f16, tag="x")
                nc.sync.dma_start(xt[:], d_xseq[c])
                for m in range(4):
                    t = 4 * c + m
                    if t >= T:
                        break
                    r0 = 32 * m
                    # Stage-interleaved emission: per-engine queues must
                    # alternate groups (sigA, sigB, tanhA, tanhB ...) so the
                    # two chains overlap; grouping by g would serialize them
                    # on the in-order engine queues.
                    ws = [int(sched[g, t]) for g in range(NG)]
                    rz, nb, gs, u, v, nn, d, ee = ([None] * NG for _ in
                                                   range(8))
                    for g in range(NG):
                        w, c0 = ws[g], g * W
                        rz[g] = pscon.tile([128, 2, W], f32, tag=f"rz{g}",
                                           name=f"rz{g}")
                        nb[g] = pscon.tile([128, 2, W], f32, tag=f"nb{g}",
                                           name=f"nb{g}")
                        xs = xt[r0:r0 + 8, c0:c0 + w]
                        # x-side first (no h dependency)
                        nc.tensor.matmul(rz[g][:, 0, 0:w],
                                         wx[r0:r0 + 8, 0:128],
                                         xs, start=True, stop=False,
                                         tile_position=(r0, 0))
                        nc.tensor.matmul(rz[g][:, 1, 0:w],
                                         wx[r0:r0 + 8, 128:256],
                                         xs, start=True, stop=False,
                                         tile_position=(r0, 0))
                        nc.tensor.matmul(nb[g][:, 0, 0:w],
                                         wx[r0:r0 + 8, 256:384],
                                         xs, start=True, stop=True,
                                         tile_position=(r0, 0))
                    for g in range(NG):
                        w = ws[g]
                        hs = h[g][:, 0:w]
                        nc.tensor.matmul(rz[g][:, 0, 0:w], whh[:, 0:128], hs,
                                         start=False, stop=True)
                        nc.tensor.matmul(rz[g][:, 1, 0:w], whh[:, 128:256],
                                         hs, start=False, stop=True)
                        nc.tensor.matmul(nb[g][:, 1, 0:w], whh[:, 256:384],
                                         hs, start=True, stop=True)
                    for g in range(NG):
                        w = ws[g]
                        gs[g] = gw.tile([128, 2, W], f16, tag=f"gs{g}",
                                        name=f"gs{g}")
                        nc.scalar.activation(gs[g][:, :, 0:w],
                                             rz[g][:, :, 0:w], Act.Sigmoid)
                    for g in range(NG):
                        w = ws[g]
                        u[g] = gw.tile([128, W], f16, tag=f"u{g}",
                                       name=f"u{g}")
                        v[g] = gw.tile([128, W], f16, tag=f"v{g}",
                                       name=f"v{g}")
                        nc.vector.scalar_tensor_tensor(
                            u[g][:, 0:w], nb[g][:, 1, 0:w], bhhn[:],
                            gs[g][:, 0, 0:w], Alu.add, Alu.mult)
                        nc.vector.tensor_add(v[g][:, 0:w], u[g][:, 0:w],
                                             nb[g][:, 0, 0:w])
                    for g in range(NG):
                        w = ws[g]
                        nn[g] = gw.tile([128, W], f16, tag=f"nn{g}",
                                        name=f"nn{g}")
                        nc.scalar.activation(nn[g][:, 0:w], v[g][:, 0:w],
                                             Act.Tanh)
                    for g in range(NG):
                        w = ws[g]
                        d[g] = gw.tile([128, W], f16, tag=f"d{g}",
                                       name=f"d{g}")
                        ee[g] = gw.tile([128, W], f16, tag=f"e{g}",
                                        name=f"e{g}")
                        nc.vector.tensor_sub(d[g][:, 0:w], nn[g][:, 0:w],
                                             h[g][:, 0:w])
                        nc.vector.tensor_mul(ee[g][:, 0:w], gs[g][:, 1, 0:w],
                                             d[g][:, 0:w])
                    for g in range(NG):
                        w = ws[g]
                        nc.vector.tensor_add(h[g][:, 0:w], h[g][:, 0:w],
                                             ee[g][:, 0:w])

        # ---- transition: hcp16[:, j*EPB+bb] = h_con of (j, ev[bb]) ----
        hcp16 = state.tile([128, SEQ], f16, tag="hcp16", name="hcp16")
        with tc.tile_pool(name=f"pstr{rep}", bufs=2, space="PSUM") as pstr, \
             tc.tile_pool(name=f"pshc{rep}", bufs=1, space="PSUM") as pshc, \
             tc.tile_pool(name=f"tr{rep}", bufs=1) as tr:
            ident = tr.tile([128, 128], f32)
            nc.gpsimd.dma_start(ident[:], d_ident[:])
            h32 = tr.tile([128, SEQ], f32)
            for g in range(NG):
                nc.vector.tensor_copy(h32[:, g * W:(g + 1) * W], h[g][:])
            hcpp = pshc.tile([128, SEQ], f32)
            chunks = [(0, 128), (128, 128), (256, 64)]
            for k, (off, w) in enumerate(chunks):
                tp = pstr.tile([128, 128], f32, tag="tp")
                nc.tensor.transpose(tp[0:w, :], h32[:, off:off + w], ident[:])
                ht = tr.tile([128, 128], f16, tag=f"ht{k}")
                nc.vector.tensor_copy(ht[0:w, :], tp[0:w, :])
                pm = tr.tile([128, SEQ], f16, tag=f"pm{k}")
                nc.gpsimd.dma_start(pm[0:w, :], d_pmat[off:off + w, :])
                nc.tensor.matmul(hcpp[:], ht[0:w, :], pm[0:w, :],
                                 start=(k == 0), stop=(k == 2))
            nc.vector.tensor_copy(hcp16[:], hcpp[:])

        # ---- jet GRU ----
        with tc.tile_pool(name=f"jw{rep}", bufs=1) as jw, \
             tc.tile_pool(name=f"psjet{rep}", bufs=2, space="PSUM") as psjet, \
             tc.tile_pool(name=f"jg{rep}", bufs=2) as jg:
            wfhcp = jw.tile([128, 96], f16)
            wfhj = jw.tile([66, 96], f16)
            whhfA = jw.tile([33, 64], f16)
            whhfB = jw.tile([33, 32], f16)
            wdiff = jw.tile([33, 1], f16)
            for dst, dsrc in [(wfhcp, d_wfhcp), (wfhj, d_wfhj),
                              (whhfA, d_whhfA), (whhfB, d_whhfB),
                              (wdiff, d_wdiff)]:
                nc.gpsimd.dma_start(dst[:], dsrc[:])

            hf = jw.tile([33, EPB], f16)
            nc.vector.memset(hf[0:32, :], 0.0)
            nc.vector.memset(hf[32:33, :], 1.0)

            for j in range(J):
                cols = slice(j * EPB, (j + 1) * EPB)
                A = psjet.tile([32, 96], f32, tag="A")
                Bb = psjet.tile([32, 32], f32, tag="B")
                for g, (g0, g1) in enumerate([(0, 32), (32, 64), (64, 96)]):
                    nc.tensor.matmul(A[:, g0:g1], wfhcp[:, g0:g1],
                                     hcp16[:, cols], start=(g == 0),
                                     stop=False)
                    nc.tensor.matmul(A[:, g0:g1], wfhj[:, g0:g1],
                                     hjaug[:, cols], start=False, stop=False)
                nc.tensor.matmul(A[:, 0:32], whhfA[:, 0:32], hf[:],
                                 start=False, stop=False)
                nc.tensor.matmul(A[:, 32:64], whhfA[:, 32:64], hf[:],
                                 start=False, stop=True)
                nc.tensor.matmul(Bb[:], whhfB[:], hf[:], start=True, stop=True)

                rj = jg.tile([32, 64], f16, tag="rj")
                uj = jg.tile([32, 32], f16, tag="uj")
                vj = jg.tile([32, 32], f16, tag="vj")
                nj = jg.tile([32, 32], f16, tag="nj")
                ej = jg.tile([32, 32], f16, tag="ej")
                nc.scalar.activation(rj[:], A[:, 0:64], Act.Sigmoid)
                nc.vector.scalar_tensor_tensor(uj[:], Bb[:], 0.0, rj[:, 0:32],
                                               Alu.add, Alu.mult)
                nc.vector.tensor_add(vj[:], uj[:], A[:, 64:96])
                nc.scalar.activation(nj[:], vj[:], Act.Tanh)
                nc.vector.tensor_sub(vj[:], nj[:], hf[0:32, :])
                nc.vector.tensor_mul(ej[:], rj[:, 32:64], vj[:])
                nc.vector.tensor_add(hf[0:32, :], hf[0:32, :], ej[:])

            C = psjet.tile([1, EPB], f32, tag="C")
            nc.tensor.matmul(C[:], wdiff[:], hf[:], start=True, stop=True)
            p0 = jg.tile([1, EPB], f32, tag="p0")
            p1 = jg.tile([1, EPB], f32, tag="p1")
            nc.scalar.activation(p0[:], C[:], Act.Sigmoid)
            nc.vector.tensor_scalar(p1[:], p0[:], -1.0, 1.0, Alu.mult, Alu.add)
            nc.sync.dma_start(d_out0[:], p0[:])
            nc.sync.dma_start(d_out1[:], p1[:])

    nc.compile()
    return nc


def kernel(x_jet, x_con_kin, x_con_type, jet_mask, con_mask,
           W_jet, b_jet, emb, Wih_c, Whh_c, bih_c, bhh_c,
           Wih_f, Whh_f, bih_f, bhh_f, W_out, b_out):
    global last_results, last_nc, last_in_maps
    from concourse.bass_utils import run_bass_kernel_spmd

    args = [np.asarray(a) for a in
            (x_jet, x_con_kin, x_con_type, jet_mask, con_mask, W_jet, b_jet,
             emb, Wih_c, Whh_c, bih_c, bhh_c, Wih_f, Whh_f, bih_f, bhh_f,
             W_out, b_out)]
    (x_jet, x_con_kin, x_con_type, jet_mask, con_mask, W_jet, b_jet, emb,
     Wih_c, Whh_c, bih_c, bhh_c, Wih_f, Whh_f, bih_f, bhh_f,
     W_out, b_out) = [a.astype(np.float32) if a.dtype.kind == "f" else a
                      for a in args]

    shared, percore, events, T, sched = _prep(
        x_jet, x_con_kin, x_con_type, jet_mask, con_mask, W_jet, b_jet, emb,
        Wih_c, Whh_c, bih_c, bhh_c, Wih_f, Whh_f, bih_f, bhh_f, W_out, b_out)

    nc = _build(T, sched)

    in_maps = [{**shared, **percore[c]} for c in range(NCORES)]
    last_nc, last_in_maps = nc, in_maps
    globals()["last_T"], globals()["last_sched"] = T, sched
    res = run_bass_kernel_spmd(nc, in_maps, core_ids=list(range(NCORES)))
    last_results = res

    probs = np.zeros((B, 2), dtype=np.float32)
    for c in range(NCORES):
        o0 = res.results[c]["out0"][0]
        o1 = res.results[c]["out1"][0]
        probs[events[c], 0] = o0
        probs[events[c], 1] = o1
    return probs
